# revision 14
# baseline (speedup 1.0000x reference)
"""Trainium2 Bass kernel for nn_CausalSelfAttention_17368847745133.

Strategy (8 NeuronCores): hybrid shard — core (b, g) = batch b in 0..3,
head-group g in 0..1 (8 heads each, Megatron-style column/row parallel
c_attn / c_proj).  Host passes x[b].T so all device matmuls run without
transposes:

  qT/kT  [512,2048] = (W_q|k slice).T-free form: out = W.T? no ->
         matmul(out=qT_tile, lhsT=W_attn_slice, rhs=xT)    (transposed proj)
  V      [2048,512] natural: matmul(lhsT=xT_tile, rhs=W_v_slice)
  S^T    [k,q] per head: matmul(lhsT=kT_head, rhs=qT_head) (contraction d=64)
  P^T    = exp((S^T + causal_mask) * 1/8)   (ACT; masked lanes underflow to 0)
  U'     [65,q] = matmul(lhsT=[V_head|ones], rhs=P^T): rows 0-63 = unnorm y^T,
         row 64 = softmax denominator
  y^T    = U'[0:64] * bcast(qm / U'[64]) + ypad * (1-qm)   (l-masking as data)
  out^T  [1024,2048] partial: matmul(lhsT=W_proj_rows, rhs=y^T); host sums the
         two group partials, transposes, adds b_proj.

Rows q >= l[b] reproduce the reference exactly: there the additive -1e8 mask
makes all logits quantize to -1e8 so softmax is uniform -> y = mean_k v.
We compute that as ypad = (1/2048) * sum_k v via a small matmul chain and
blend it in with the 0/1 column masks qm/qmn (built on host from l).

Matmul inputs are bitcast to float32r (full-rate fp32 mode, ~1e-4 rel err).
"""

import numpy as np

import concourse.bass as bass
import concourse.mybir as mybir
import concourse.tile as tile
from concourse import bacc
from concourse.bass_utils import run_bass_kernel_spmd

P = 128
B, T, C = 4, 2048, 1024
H, D = 16, 64
G = 2            # head groups (cores per batch)
HPG = H // G     # 8 heads per core
CG = HPG * D     # 512 channels per group
NEG = -1e8
F32 = mybir.dt.float32
F32R = mybir.dt.float32r
BF16 = mybir.dt.bfloat16

_CACHED_NC = None


def _f(ap):
    """view a float32r AP as plain fp32 (for DMA byte moves)."""
    return ap.bitcast(F32)


def build_nc(debug=False):
    nc = bacc.Bacc(trn_type="TRN2", target_bir_lowering=False)

    xT = nc.dram_tensor("xT", [C, T], F32, kind="ExternalInput")
    wq = nc.dram_tensor("wq", [P, 8, CG], F32, kind="ExternalInput")
    wk = nc.dram_tensor("wk", [P, 8, CG], F32, kind="ExternalInput")
    wv = nc.dram_tensor("wv", [P, 8, CG], F32, kind="ExternalInput")
    wp = nc.dram_tensor("wp", [P, 4, C], F32, kind="ExternalInput")
    qmn = nc.dram_tensor("qmn", [P, T], F32, kind="ExternalInput")  # 1 - qm
    m128 = nc.dram_tensor("m128", [P, P], F32, kind="ExternalInput")
    oT = nc.dram_tensor("oT", [C, T], F32, kind="ExternalOutput")
    if debug:
        d_qT = nc.dram_tensor("d_qT", [P, 4, T], F32, kind="ExternalOutput")
        d_kT = nc.dram_tensor("d_kT", [P, 4, T], F32, kind="ExternalOutput")
        d_V = nc.dram_tensor("d_V", [P, 16, HPG, D + 1], F32, kind="ExternalOutput")
        d_yT = nc.dram_tensor("d_yT", [P, 4, T], F32, kind="ExternalOutput")
        d_S = nc.dram_tensor("d_S", [P, 512], F32, kind="ExternalOutput")
        d_PT = nc.dram_tensor("d_PT", [P, 512], F32, kind="ExternalOutput")
        d_U = nc.dram_tensor("d_U", [D + 1, 512], F32, kind="ExternalOutput")

    with tile.TileContext(nc) as tc:
        with tc.tile_pool(name="big", bufs=1) as big, \
             tc.tile_pool(name="qk", bufs=1) as qkpool, \
             tc.tile_pool(name="vp", bufs=1) as vpool, \
             tc.tile_pool(name="w", bufs=1) as wpool, \
             tc.tile_pool(name="pt", bufs=4) as ptpool, \
             tc.tile_pool(name="misc", bufs=1) as misc, \
             tc.tile_pool(name="norm", bufs=2) as norm, \
             tc.tile_pool(name="ob", bufs=3) as obpool, \
             tc.tile_pool(name="stage", bufs=2) as stage, \
             tc.tile_pool(name="rdram", bufs=3, space="DRAM") as rdram, \
             tc.tile_pool(name="psS", bufs=3, space="PSUM") as psS, \
             tc.tile_pool(name="psU", bufs=3, space="PSUM") as psU, \
             tc.tile_pool(name="psB", bufs=2, space="PSUM") as psB:

            # ---- constant/context loads ----
            qmn_sb = misc.tile([P, T], F32, tag="qmn")
            m128_sb = misc.tile([P, P], F32, tag="m128")
            uni_sb = misc.tile([P, 2], BF16, tag="uni")
            nc.sync.dma_start(qmn_sb, qmn[:])
            nc.sync.dma_start(m128_sb, m128[:])
            nc.vector.memset(uni_sb, 1.0 / T)

            # ---- Phase B: QKV projections (two T-halves to save SBUF) ----
            qT_sb = qkpool.tile([P, 4, T], F32R, tag="qT")
            kT_sb = qkpool.tile([P, 4, T], F32R, tag="kT")
            V_sb = vpool.tile([P, 16, HPG, D + 1], BF16, tag="V")

            TH = T // 2
            for th in range(2):
                xT_sb = big.tile([P, 8, TH], F32R, tag="big",
                                 name=f"xT_{th}")
                for ct in range(8):
                    st = stage.tile([P, 1024], F32, tag="stage")
                    nc.sync.dma_start(
                        st, xT[ct * P:(ct + 1) * P, th * TH:(th + 1) * TH])
                    nc.vector.tensor_copy(xT_sb[:, ct], st)

                for side, (wdram, dst) in enumerate([(wq, qT_sb),
                                                     (wk, kT_sb)]):
                    w_sb = wpool.tile([P, 8, CG], F32R, tag="w",
                                      name=f"w_{th}_{side}")
                    wflat = w_sb.rearrange("p a b -> p (a b)")
                    dflat = wdram[:].rearrange("p a b -> p (a b)")
                    for ch in range(4):
                        st = stage.tile([P, 1024], F32, tag="stage")
                        nc.sync.dma_start(st, dflat[:, ch * 1024:(ch + 1) * 1024])
                        nc.vector.tensor_copy(
                            wflat[:, ch * 1024:(ch + 1) * 1024], st)
                    for mt in range(4):
                        for nb in range(2):
                            ps = psB.tile([P, 512], F32, tag="psB")
                            for kt in range(8):
                                nc.tensor.matmul(
                                    ps,
                                    w_sb[:, kt, mt * P:(mt + 1) * P],
                                    xT_sb[:, kt, nb * 512:(nb + 1) * 512],
                                    start=(kt == 0), stop=(kt == 7))
                            nc.vector.tensor_copy(
                                dst[:, mt,
                                    th * TH + nb * 512:th * TH + (nb + 1) * 512],
                                ps)

                wv_sb = wpool.tile([P, 8, CG], F32R, tag="w",
                                   name=f"wv_{th}")
                wvflat = wv_sb.rearrange("p a b -> p (a b)")
                dvflat = wv[:].rearrange("p a b -> p (a b)")
                for ch in range(4):
                    st = stage.tile([P, 1024], F32, tag="stage")
                    nc.sync.dma_start(st, dvflat[:, ch * 1024:(ch + 1) * 1024])
                    nc.vector.tensor_copy(
                        wvflat[:, ch * 1024:(ch + 1) * 1024], st)
                for tt in range(8):
                    ps = psB.tile([P, 512], F32, tag="psB")
                    for kt in range(8):
                        nc.tensor.matmul(
                            ps,
                            xT_sb[:, kt, tt * P:(tt + 1) * P],
                            wv_sb[:, kt, :],
                            start=(kt == 0), stop=(kt == 7))
                    tg = th * 8 + tt
                    # scatter [128, 8, 64] into the 65-wide per-head slots
                    nc.scalar.copy(V_sb[:, tg, :, 0:D],
                                   ps.rearrange("p (h d) -> p h d", h=HPG))
                    nc.vector.memset(V_sb[:, tg, :, D:D + 1], 1.0)

            if debug:
                nc.sync.dma_start(d_qT[:], _f(qT_sb))
                nc.sync.dma_start(d_kT[:], _f(kT_sb))
                nc.gpsimd.dma_start(d_V[:], V_sb)

            # ---- Phase C: attention per head-pair ----
            yT_sb = big.tile([P, 4, T], BF16, tag="big")  # reuses the xT slot

            for hp in range(4):
                ypadB = {}
                for parity in range(2):
                    h = 2 * hp + parity
                    # ypad_h = (1/T) * sum_k v  via 16 N=1 fp32 matmuls
                    psPad = psU.tile([D + 1, 512], F32, tag="psU")
                    for kt in range(16):
                        nc.tensor.matmul(
                            psPad[:, 0:1],
                            V_sb[:, kt, h, :],
                            uni_sb[:, 0:1],
                            start=(kt == 0), stop=(kt == 15))
                    yb = norm.tile([D, 512], F32, tag="ypadB")
                    nc.vector.tensor_copy(
                        yb, psPad[0:D, 0:1].to_broadcast([D, 512]))
                    ypadB[parity] = yb

                for j in range(4):
                    nkt = 4 * (j + 1)
                    Upr = [psU.tile([D + 1, 512], F32, tag="psU",
                                    name=f"U_{hp}_{j}_{par}")
                           for par in range(2)]

                    def s_exp(kt, j=j, hp=hp):
                        out = []
                        for parity in range(2):
                            h = 2 * hp + parity
                            p0 = parity * D
                            ct = h // 2
                            dlt = 128 * kt - 512 * j
                            c0 = max(dlt, 0)
                            ss = psS.tile([P, 512], F32, tag="psS")
                            nc.tensor.matmul(
                                ss[:, c0:512],
                                kT_sb[p0:p0 + D, ct, kt * P:(kt + 1) * P],
                                qT_sb[p0:p0 + D, ct, 512 * j + c0:512 * (j + 1)],
                                start=True, stop=True)
                            if dlt >= 0:
                                nc.vector.tensor_add(
                                    out=ss[:, c0:c0 + P],
                                    in0=ss[:, c0:c0 + P], in1=m128_sb)
                            if debug and hp == 0 and parity == 0 \
                                    and j == 0 and kt == 0:
                                dbg = stage.tile([P, 512], F32, tag="dbg")
                                nc.vector.tensor_copy(dbg, ss)
                                nc.sync.dma_start(d_S[:], dbg)
                            pt = ptpool.tile([P, 512], BF16, tag="pt")
                            if c0 > 0:
                                nc.vector.memset(pt[:, 0:c0], 0.0)
                            nc.scalar.activation(
                                pt[:, c0:512], ss[:, c0:512],
                                mybir.ActivationFunctionType.Exp,
                                bias=0.0, scale=1.0 / np.sqrt(D))
                            if debug and hp == 0 and parity == 0 \
                                    and j == 0 and kt == 0:
                                nc.gpsimd.dma_start(d_PT[:], pt)
                            out.append(pt)
                        return out

                    def pv(kt, pts, hp=hp):
                        for parity in range(2):
                            h = 2 * hp + parity
                            nc.tensor.matmul(
                                Upr[parity],
                                V_sb[:, kt, h, :],
                                pts[parity],
                                start=(kt == 0), stop=(kt == nkt - 1))

                    prev = None
                    for kt in range(nkt):
                        cur = s_exp(kt)
                        if prev is not None:
                            pv(kt - 1, prev)
                        prev = cur
                    pv(nkt - 1, prev)

                    if debug and hp == 0 and j == 0:
                        dbgu = stage.tile([D + 1, 512], F32, tag="dbg")
                        nc.vector.tensor_copy(dbgu, Upr[0])
                        nc.sync.dma_start(d_U[:], dbgu)
                    # normalize + l-blend -> yT.  All elementwise ops keep
                    # in/out on identical partition ranges (HW requirement);
                    # the odd head's result is moved to partitions 64-127 by
                    # an SBUF-to-SBUF DMA at the end.
                    for parity in range(2):
                        h = 2 * hp + parity
                        p0 = parity * D
                        ct = h // 2
                        U = Upr[parity]
                        blk = slice(512 * j, 512 * (j + 1))
                        # rc lives at partition 64 (same as U's denom row)
                        rcf = norm.tile([P, 512], F32, tag="rc")
                        rc = rcf[D:D + 1, :]
                        nc.vector.reciprocal(rc, U[D:D + 1, :])
                        rcmf = norm.tile([P, 512], F32, tag="rcm")
                        rcm = rcmf[D:D + 1, :]
                        nc.vector.tensor_mul(
                            out=rcm, in0=rc, in1=qmn_sb[D:D + 1, blk])
                        nc.vector.tensor_tensor(
                            rc, rc, rcm, mybir.AluOpType.subtract)
                        rb = norm.tile([D, 512], F32, tag="rb")
                        rcd = rdram.tile([1, 512], F32, tag="rcd")
                        nc.sync.dma_start(rcd, rc)
                        rc_bcast = bass.AP(
                            tensor=rcd.tensor, offset=rcd.offset,
                            ap=[[0, D]] + list(rcd.ap[1:]))
                        nc.sync.dma_start(rb, rc_bcast)
                        ysf = norm.tile([D, 512], F32, tag="ysf")
                        nc.vector.tensor_mul(out=ysf, in0=U[0:D, :], in1=rb)
                        t3 = norm.tile([D, 512], F32, tag="t3")
                        nc.vector.tensor_mul(
                            out=t3, in0=ypadB[parity],
                            in1=qmn_sb[0:D, blk])
                        if parity == 0:
                            nc.vector.tensor_add(
                                out=yT_sb[0:D, ct, blk], in0=ysf, in1=t3)
                        else:
                            ysb = norm.tile([D, 512], BF16, tag="ysb")
                            nc.vector.tensor_add(out=ysb, in0=ysf, in1=t3)
                            nc.sync.dma_start(yT_sb[D:P, ct, blk], ysb)

            # ---- Phase D: output projection (row-parallel partial) ----
            if debug:
                nc.gpsimd.dma_start(d_yT[:], yT_sb)
            wp_v = wpool.tile([P, 4, C], BF16, tag="w")
            nc.gpsimd.dma_start(wp_v, wp[:])
            for mt in range(8):
                for qb in range(4):
                    ps = psB.tile([P, 512], F32, tag="psB")
                    for ct in range(4):
                        nc.tensor.matmul(
                            ps,
                            wp_v[:, ct, mt * P:(mt + 1) * P],
                            yT_sb[:, ct, qb * 512:(qb + 1) * 512],
                            start=(ct == 0), stop=(ct == 3))
                    ot = obpool.tile([P, 512], F32, tag="ob")
                    nc.scalar.copy(ot, ps)
                    nc.sync.dma_start(
                        oT[mt * P:(mt + 1) * P, qb * 512:(qb + 1) * 512], ot)

    nc.compile()
    return nc


def _prep_inputs(x, l, W_attn, b_attn, W_proj, b_proj):
    x = np.asarray(x, dtype=np.float32)
    W_attn = np.asarray(W_attn, dtype=np.float32)
    W_proj = np.asarray(W_proj, dtype=np.float32)
    lv = np.asarray(l).astype(np.int64)

    m128 = np.where(np.arange(P)[:, None] > np.arange(P)[None, :],
                    np.float32(NEG), np.float32(0.0)).astype(np.float32)

    in_maps = []
    for b in range(B):
        xTb = np.ascontiguousarray(x[b].T)
        lb = int(np.clip(lv[b], 0, T))
        qrow = (np.arange(T) < lb).astype(np.float32)
        qmn = np.ascontiguousarray(np.broadcast_to(1.0 - qrow, (P, T))
                                   ).astype(np.float32)
        for g in range(2):
            cs = slice(g * CG, (g + 1) * CG)
            wqg = np.ascontiguousarray(
                W_attn[:, 0 * C:1 * C][:, cs].reshape(8, P, CG).transpose(1, 0, 2))
            wkg = np.ascontiguousarray(
                W_attn[:, 1 * C:2 * C][:, cs].reshape(8, P, CG).transpose(1, 0, 2))
            wvg = np.ascontiguousarray(
                W_attn[:, 2 * C:3 * C][:, cs].reshape(8, P, CG).transpose(1, 0, 2))
            wpg = np.ascontiguousarray(
                W_proj[cs, :].reshape(4, P, C).transpose(1, 0, 2))
            in_maps.append({
                "xT": xTb, "wq": wqg, "wk": wkg, "wv": wvg, "wp": wpg,
                "qmn": qmn, "m128": m128,
            })
    return in_maps


def kernel(x, l, W_attn, b_attn, W_proj, b_proj, _want_profile=False):
    global _CACHED_NC
    if _CACHED_NC is None:
        _CACHED_NC = build_nc()
    nc = _CACHED_NC

    b_attn = np.asarray(b_attn, dtype=np.float32)
    b_proj = np.asarray(b_proj, dtype=np.float32)
    assert not np.any(b_attn), "nonzero b_attn not supported by this kernel"

    in_maps = _prep_inputs(x, l, W_attn, b_attn, W_proj, b_proj)
    res = run_bass_kernel_spmd(nc, in_maps, core_ids=list(range(8)),
                               trace=_want_profile)

    out = np.empty((B, T, C), dtype=np.float32)
    for b in range(B):
        acc = res.results[2 * b]["oT"] + res.results[2 * b + 1]["oT"]
        out[b] = acc.T + b_proj[None, :]
    if _want_profile:
        return out, res
    return out


# revision 15
# speedup vs baseline: 1.5042x; 1.5042x over previous
"""Trainium2 Bass kernel for nn_CausalSelfAttention_17368847745133.

Sharding (8 NeuronCores): core (b, g) = batch b in 0..3 x head-group g in
0..1 (8 heads each; Megatron column/row-parallel c_attn / c_proj).  The host
passes x[b].T so every device matmul runs transpose-free:

  qT/kT [512,2048] : matmul(lhsT=W_q|k slice, rhs=xT)      (transposed proj)
  V     [2048,512] : matmul(lhsT=xT tile, rhs=W_v slice)   (natural layout)
  S^T   [k,q]      : matmul(lhsT=kT head, rhs=qT head)     (d=64 contraction,
                     head pairs packed on PE row-groups 0-63 / 64-127)
  P^T   = exp((S^T + causal_mask) / 8)    masked lanes underflow to exact 0
  U'    [65,q]     : matmul(lhsT=[V_head|ones], rhs=P^T)   row 64 = denom
  y^T   = U'[0:64] * bcast(qm / denom) + ypad * (1 - qm)
  oT    [1024,2048]: matmul(lhsT=W_proj rows, rhs=y^T); host sums the two
                     group partials, transposes, adds b_proj.

Rows q >= l[b] reproduce the reference exactly: the reference's additive
-1e8 mask makes every logit in those rows quantize to -1e8, so its softmax
is exactly uniform and y = mean_k v.  We compute ypad = (1/2048) sum_k v
once per head and blend it by the 0/1 column mask (built on host from l).
All matmuls run in bf16 (inputs cast on load); softmax statistics and the
normalization stay fp32.  Reciprocals are batched for all 64 (head, block)
rows into one 32-partition DVE op.
"""

import numpy as np

import concourse.bass as bass
import concourse.mybir as mybir
import concourse.tile as tile
from concourse import bacc
from concourse.bass_utils import run_bass_kernel_spmd

P = 128
B, T, C = 4, 2048, 1024
H, D = 16, 64
G = 2
HPG = H // G     # 8 heads per core
CG = HPG * D     # 512 channels per group
NEG = -1e8
F32 = mybir.dt.float32
BF16 = mybir.dt.bfloat16
SCALE = 0.125    # 1/sqrt(64)

_CACHED_NC = None


def build_nc(debug=False):
    nc = bacc.Bacc(trn_type="TRN2", target_bir_lowering=False)

    xT = nc.dram_tensor("xT", [C, T], F32, kind="ExternalInput")
    wq = nc.dram_tensor("wq", [P, 8, CG], F32, kind="ExternalInput")
    wk = nc.dram_tensor("wk", [P, 8, CG], F32, kind="ExternalInput")
    wv = nc.dram_tensor("wv", [P, 8, CG], F32, kind="ExternalInput")
    wp = nc.dram_tensor("wp", [P, 4, C], F32, kind="ExternalInput")
    qmn = nc.dram_tensor("qmn", [P, T], F32, kind="ExternalInput")   # 1-qm
    qmA = nc.dram_tensor("qmA", [32, 512], F32, kind="ExternalInput")
    m128 = nc.dram_tensor("m128", [P, P], F32, kind="ExternalInput")
    oT = nc.dram_tensor("oT", [C, T], F32, kind="ExternalOutput")
    if debug:
        d_yT = nc.dram_tensor("d_yT", [P, 4, T], F32, kind="ExternalOutput")

    with tile.TileContext(nc) as tc:
        with tc.tile_pool(name="big", bufs=1) as big, \
             tc.tile_pool(name="qk", bufs=1) as qkpool, \
             tc.tile_pool(name="vp", bufs=1) as vpool, \
             tc.tile_pool(name="w", bufs=2) as wpool, \
             tc.tile_pool(name="pt", bufs=4) as ptpool, \
             tc.tile_pool(name="misc", bufs=1) as misc, \
             tc.tile_pool(name="norm", bufs=2) as norm, \
             tc.tile_pool(name="ob", bufs=3) as obpool, \
             tc.tile_pool(name="rdram", bufs=2, space="DRAM") as rdram, \
             tc.tile_pool(name="psS", bufs=2, space="PSUM") as psS, \
             tc.tile_pool(name="psU", bufs=3, space="PSUM") as psU:

            # ---- constants ----
            qmn_sb = misc.tile([P, T], F32, tag="qmn")
            m128_sb = misc.tile([P, P], F32, tag="m128")
            qmA_sb = misc.tile([32, 512], F32, tag="qmA")
            uni_sb = misc.tile([P, 2], BF16, tag="uni")
            nc.sync.dma_start(qmn_sb, qmn[:])
            nc.sync.dma_start(m128_sb, m128[:])
            nc.sync.dma_start(qmA_sb, qmA[:])
            nc.vector.memset(uni_sb, 1.0 / T)
            # all (head, block) softmax denominators, gathered by small DMAs
            den_sb = misc.tile([32, 512], F32, tag="den")

            # ---- Phase B: QKV projections (bf16, casting DMA loads) ----
            xT_bf = big.tile([P, 8, T], BF16, tag="big")
            for ct in range(8):
                nc.gpsimd.dma_start(xT_bf[:, ct], xT[ct * P:(ct + 1) * P, :])

            qT_sb = qkpool.tile([P, 4, T], BF16, tag="qT")
            kT_sb = qkpool.tile([P, 4, T], BF16, tag="kT")
            V_sb = vpool.tile([P, 16, HPG, D + 1], BF16, tag="V")

            for side, (wdram, dst) in enumerate([(wq, qT_sb), (wk, kT_sb)]):
                w_sb = wpool.tile([P, 8, CG], BF16, tag="w", name=f"w{side}")
                nc.gpsimd.dma_start(w_sb, wdram[:])
                for mt in range(4):
                    for nbh in range(2):
                        ps = psS.tile([P, 2, 512], F32, tag="psS")
                        for kt in range(8):
                            for nb2 in range(2):
                                nc.tensor.matmul(
                                    ps[:, nb2],
                                    w_sb[:, kt, mt * P:(mt + 1) * P],
                                    xT_bf[:, kt,
                                          nbh * 1024 + nb2 * 512:
                                          nbh * 1024 + (nb2 + 1) * 512],
                                    start=(kt == 0), stop=(kt == 7))
                        nc.vector.tensor_copy(
                            dst[:, mt, nbh * 1024:(nbh + 1) * 1024],
                            ps.rearrange("p a b -> p (a b)"))

            wv_sb = wpool.tile([P, 8, CG], BF16, tag="w", name="wvs")
            nc.gpsimd.dma_start(wv_sb, wv[:])
            for tt in range(16):
                ps = psU.tile([P, 512], F32, tag="psU", name=f"psV{tt}")
                for kt in range(8):
                    nc.tensor.matmul(
                        ps,
                        xT_bf[:, kt, tt * P:(tt + 1) * P],
                        wv_sb[:, kt, :],
                        start=(kt == 0), stop=(kt == 7))
                nc.scalar.copy(V_sb[:, tt, :, 0:D],
                               ps.rearrange("p (h d) -> p h d", h=HPG))
                nc.vector.memset(V_sb[:, tt, :, D:D + 1], 1.0)

            # ---- Phase C: attention (head pairs on PE row groups) ----
            yT_sb = big.tile([P, 4, T], BF16, tag="big")   # reuses xT slot
            # per-head ypad rows, stored at the head's partition range
            ypadA = misc.tile([P, 4, 512], F32, tag="ypadA")

            for hp in range(4):
                for parity in range(2):
                    h = 2 * hp + parity
                    psPad = psU.tile([D + 1, 512], F32, tag="psU",
                                     name=f"pad{h}")
                    for kt in range(16):
                        nc.tensor.matmul(
                            psPad[:, 0:1],
                            V_sb[:, kt, h, :],
                            uni_sb[:, 0:1],
                            start=(kt == 0), stop=(kt == 15))
                    if parity == 0:
                        nc.vector.tensor_copy(
                            ypadA[0:D, hp, :],
                            psPad[0:D, 0:1].to_broadcast([D, 512]))
                    else:
                        yptmp = norm.tile([D, 512], F32, tag="yptmp")
                        nc.vector.tensor_copy(
                            yptmp, psPad[0:D, 0:1].to_broadcast([D, 512]))
                        nc.sync.dma_start(ypadA[D:P, hp, :], yptmp)

                for j in range(4):
                    nkt = 4 * (j + 1)
                    Upr = [psU.tile([D + 1, 512], F32, tag="psU",
                                    name=f"U_{hp}_{j}_{par}")
                           for par in range(2)]

                    def s_exp(kt, j=j, hp=hp):
                        dlt = 128 * kt - 512 * j
                        c0 = max(dlt, 0)
                        ss = psS.tile([P, 2, 512], F32, tag="psS")
                        for parity in range(2):
                            p0 = parity * D
                            nc.tensor.matmul(
                                ss[:, parity, c0:512],
                                kT_sb[p0:p0 + D, hp, kt * P:(kt + 1) * P],
                                qT_sb[p0:p0 + D, hp,
                                      512 * j + c0:512 * (j + 1)],
                                start=True, stop=True)
                        if dlt >= 0:
                            nc.vector.tensor_add(
                                out=ss[:, :, c0:c0 + P],
                                in0=ss[:, :, c0:c0 + P],
                                in1=m128_sb[:, None, :].to_broadcast(
                                    [P, 2, P]))
                        pt = ptpool.tile([P, 2, 512], BF16, tag="pt")
                        if c0 > 0:
                            nc.vector.memset(pt[:, :, 0:c0], 0.0)
                        nc.scalar.activation(
                            pt[:, :, c0:512], ss[:, :, c0:512],
                            mybir.ActivationFunctionType.Exp,
                            bias=0.0, scale=SCALE)
                        return pt

                    def pv(kt, pt, hp=hp):
                        for parity in range(2):
                            h = 2 * hp + parity
                            nc.tensor.matmul(
                                Upr[parity],
                                V_sb[:, kt, h, :],
                                pt[:, parity, :],
                                start=(kt == 0), stop=(kt == nkt - 1))

                    prev = None
                    for kt in range(nkt):
                        cur = s_exp(kt)
                        if prev is not None:
                            pv(kt - 1, prev)
                        prev = cur
                    pv(nkt - 1, prev)

                    # stash unnormalized y and the denominator row
                    for parity in range(2):
                        U = Upr[parity]
                        blk = slice(512 * j, 512 * (j + 1))
                        r = (hp * 2 + parity) * 4 + j
                        dtf = norm.tile([P, 512], F32, tag="dt")
                        nc.vector.tensor_copy(dtf[D:D + 1, :], U[D:D + 1, :])
                        nc.sync.dma_start(den_sb[r:r + 1, :],
                                          dtf[D:D + 1, :])
                        if parity == 0:
                            nc.vector.tensor_copy(yT_sb[0:D, hp, blk],
                                                  U[0:D, :])
                        else:
                            ytmp = norm.tile([D, 512], BF16, tag="ytmp")
                            nc.vector.tensor_copy(ytmp, U[0:D, :])
                            nc.sync.dma_start(yT_sb[D:P, hp, blk], ytmp)

            # ---- Phase C2: batched normalization + l-blend ----
            denq = misc.tile([32, 512], F32, tag="denq")
            nc.vector.reciprocal(denq, den_sb)
            nc.vector.tensor_mul(out=denq, in0=denq, in1=qmA_sb)
            dend = rdram.tile([32, 512], F32, tag="dend")
            nc.sync.dma_start(dend, denq)

            for hp in range(4):
                for j in range(4):
                    blk = slice(512 * j, 512 * (j + 1))
                    rb = norm.tile([P, 512], F32, tag="rb")
                    for parity in range(2):
                        r = (hp * 2 + parity) * 4 + j
                        row = dend[r:r + 1, :]
                        src = bass.AP(
                            tensor=row.tensor, offset=row.offset,
                            ap=[[0, D]] + list(row.ap[1:]))
                        nc.sync.dma_start(rb[parity * D:(parity + 1) * D, :],
                                          src)
                    t3 = norm.tile([P, 512], F32, tag="t3")
                    nc.vector.tensor_mul(
                        out=t3, in0=ypadA[:, hp, :], in1=qmn_sb[:, blk])
                    ys = yT_sb[:, hp, blk]
                    nc.vector.tensor_mul(out=ys, in0=ys, in1=rb)
                    nc.vector.tensor_add(out=ys, in0=ys, in1=t3)

            if debug:
                nc.gpsimd.dma_start(d_yT[:], yT_sb)

            # ---- Phase D: output projection ----
            wp_v = wpool.tile([P, 4, C], BF16, tag="w", name="wpv")
            nc.gpsimd.dma_start(wp_v, wp[:])
            for mt in range(8):
                psa = psS.tile([P, 2, 512], F32, tag="psS", name=f"po{mt}a")
                psb = psS.tile([P, 2, 512], F32, tag="psS", name=f"po{mt}b")
                outs = [psa[:, 0], psa[:, 1], psb[:, 0], psb[:, 1]]
                for ct in range(4):
                    for qb in range(4):
                        nc.tensor.matmul(
                            outs[qb],
                            wp_v[:, ct, mt * P:(mt + 1) * P],
                            yT_sb[:, ct, qb * 512:(qb + 1) * 512],
                            start=(ct == 0), stop=(ct == 3))
                for half, pp in enumerate([psa, psb]):
                    ot = obpool.tile([P, 1024], F32, tag="ob")
                    nc.scalar.copy(ot, pp.rearrange("p a b -> p (a b)"))
                    nc.sync.dma_start(
                        oT[mt * P:(mt + 1) * P,
                           half * 1024:(half + 1) * 1024], ot)

    nc.compile()
    return nc


def _prep_inputs(x, l, W_attn, b_attn, W_proj, b_proj):
    x = np.asarray(x, dtype=np.float32)
    W_attn = np.asarray(W_attn, dtype=np.float32)
    W_proj = np.asarray(W_proj, dtype=np.float32)
    lv = np.asarray(l).astype(np.int64)

    m128 = np.where(np.arange(P)[:, None] > np.arange(P)[None, :],
                    np.float32(NEG), np.float32(0.0)).astype(np.float32)

    in_maps = []
    for b in range(B):
        xTb = np.ascontiguousarray(x[b].T)
        lb = int(np.clip(lv[b], 0, T))
        qrow = (np.arange(T) < lb).astype(np.float32)
        qmn = np.ascontiguousarray(np.broadcast_to(1.0 - qrow, (P, T))
                                   ).astype(np.float32)
        qmA = np.empty((32, 512), dtype=np.float32)
        for r in range(32):
            j = r % 4
            qmA[r] = qrow[512 * j:512 * (j + 1)]
        for g in range(2):
            cs = slice(g * CG, (g + 1) * CG)
            wqg = np.ascontiguousarray(
                W_attn[:, 0:C][:, cs].reshape(8, P, CG).transpose(1, 0, 2))
            wkg = np.ascontiguousarray(
                W_attn[:, C:2 * C][:, cs].reshape(8, P, CG).transpose(1, 0, 2))
            wvg = np.ascontiguousarray(
                W_attn[:, 2 * C:3 * C][:, cs].reshape(8, P, CG).transpose(1, 0, 2))
            wpg = np.ascontiguousarray(
                W_proj[cs, :].reshape(4, P, C).transpose(1, 0, 2))
            in_maps.append({
                "xT": xTb, "wq": wqg, "wk": wkg, "wv": wvg, "wp": wpg,
                "qmn": qmn, "qmA": qmA, "m128": m128,
            })
    return in_maps


def kernel(x, l, W_attn, b_attn, W_proj, b_proj, _want_profile=False):
    global _CACHED_NC
    if _CACHED_NC is None:
        _CACHED_NC = build_nc()
    nc = _CACHED_NC

    b_attn = np.asarray(b_attn, dtype=np.float32)
    b_proj = np.asarray(b_proj, dtype=np.float32)
    assert not np.any(b_attn), "nonzero b_attn not supported by this kernel"

    in_maps = _prep_inputs(x, l, W_attn, b_attn, W_proj, b_proj)
    res = run_bass_kernel_spmd(nc, in_maps, core_ids=list(range(8)),
                               trace=_want_profile)

    out = np.empty((B, T, C), dtype=np.float32)
    for b in range(B):
        acc = res.results[2 * b]["oT"] + res.results[2 * b + 1]["oT"]
        out[b] = acc.T + b_proj[None, :]
    if _want_profile:
        return out, res
    return out


# revision 16
# speedup vs baseline: 1.7084x; 1.1358x over previous
"""Trainium2 Bass kernel for nn_CausalSelfAttention_17368847745133.

Sharding (8 NeuronCores): core (b, g) = batch b in 0..3 x head-group g in
0..1 (8 heads each; Megatron column/row-parallel c_attn / c_proj).  The host
passes x[b].T so every device matmul runs transpose-free:

  qT/kT [512,2048] : matmul(lhsT=W_q|k slice, rhs=xT)      (transposed proj)
  V     [2048,512] : matmul(lhsT=xT tile, rhs=W_v slice)   (natural layout)
  S^T   [k,q]      : matmul(lhsT=kT head, rhs=qT head)     (d=64 contraction,
                     head pairs packed on PE row-groups 0-63 / 64-127)
  P^T   = exp((S^T + causal_mask) / 8)    masked lanes underflow to exact 0
  U'    [65,q]     : matmul(lhsT=[V_head|ones], rhs=P^T)   row 64 = denom
  y^T   = U'[0:64] * bcast(qm / denom) + ypad * (1 - qm)
  oT    [1024,2048]: matmul(lhsT=W_proj rows, rhs=y^T); host sums the two
                     group partials, transposes, adds b_proj.

Rows q >= l[b] reproduce the reference exactly: the reference's additive
-1e8 mask makes every logit in those rows quantize to -1e8, so its softmax
is exactly uniform and y = mean_k v.  We compute ypad = (1/2048) sum_k v
once per head and blend it by the 0/1 column mask (built on host from l).
All matmuls run in bf16 (inputs cast on load); softmax statistics and the
normalization stay fp32.  Reciprocals are batched for all 64 (head, block)
rows into one 32-partition DVE op.
"""

import numpy as np

import concourse.bass as bass
import concourse.mybir as mybir
import concourse.tile as tile
from concourse import bacc
from concourse.bass_utils import run_bass_kernel_spmd

P = 128
B, T, C = 4, 2048, 1024
H, D = 16, 64
G = 2
HPG = H // G     # 8 heads per core
CG = HPG * D     # 512 channels per group
NEG = -1e8
F32 = mybir.dt.float32
BF16 = mybir.dt.bfloat16
SCALE = 0.125    # 1/sqrt(64)

_CACHED_NC = None


def build_nc(debug=False):
    nc = bacc.Bacc(trn_type="TRN2", target_bir_lowering=False)

    xT = nc.dram_tensor("xT", [C, T], F32, kind="ExternalInput")
    wq = nc.dram_tensor("wq", [P, 8, CG], F32, kind="ExternalInput")
    wk = nc.dram_tensor("wk", [P, 8, CG], F32, kind="ExternalInput")
    wv = nc.dram_tensor("wv", [P, 8, CG], F32, kind="ExternalInput")
    wp = nc.dram_tensor("wp", [P, 4, C], F32, kind="ExternalInput")
    qmn = nc.dram_tensor("qmn", [P, T], F32, kind="ExternalInput")   # 1-qm
    qmA = nc.dram_tensor("qmA", [32, 512], F32, kind="ExternalInput")
    m01 = nc.dram_tensor("m01", [P, P], BF16, kind="ExternalInput")
    oT = nc.dram_tensor("oT", [C, T], F32, kind="ExternalOutput")
    if debug:
        d_yT = nc.dram_tensor("d_yT", [P, 4, T], F32, kind="ExternalOutput")

    with tile.TileContext(nc) as tc:
        with tc.tile_pool(name="big", bufs=1) as big, \
             tc.tile_pool(name="qk", bufs=1) as qkpool, \
             tc.tile_pool(name="vp", bufs=1) as vpool, \
             tc.tile_pool(name="w", bufs=2) as wpool, \
             tc.tile_pool(name="pt", bufs=4) as ptpool, \
             tc.tile_pool(name="misc", bufs=1) as misc, \
             tc.tile_pool(name="norm", bufs=2) as norm, \
             tc.tile_pool(name="ob", bufs=3) as obpool, \
             tc.tile_pool(name="rdram", bufs=2, space="DRAM") as rdram, \
             tc.tile_pool(name="psS", bufs=3, space="PSUM") as psS, \
             tc.tile_pool(name="psU", bufs=2, space="PSUM") as psU:

            # ---- constants ----
            qmn_sb = misc.tile([P, T], F32, tag="qmn")
            m01_sb = misc.tile([P, P], BF16, tag="m01")
            qmA_sb = misc.tile([32, 512], F32, tag="qmA")
            uni_sb = misc.tile([P, 2], BF16, tag="uni")
            nc.sync.dma_start(qmn_sb, qmn[:])
            nc.sync.dma_start(m01_sb, m01[:])
            nc.sync.dma_start(qmA_sb, qmA[:])
            nc.vector.memset(uni_sb, 1.0 / T)
            # all (head, block) softmax denominators, gathered by small DMAs
            den_sb = misc.tile([32, 512], F32, tag="den")

            # ---- Phase B: QKV projections (bf16, casting DMA loads) ----
            xT_bf = big.tile([P, 8, T], BF16, tag="big")
            for ct in range(8):
                nc.gpsimd.dma_start(xT_bf[:, ct], xT[ct * P:(ct + 1) * P, :])

            qT_sb = qkpool.tile([P, 4, T], BF16, tag="qT")
            kT_sb = qkpool.tile([P, 4, T], BF16, tag="kT")
            V_sb = vpool.tile([P, 16, HPG, D + 1], BF16, tag="V")

            w_tiles = {}
            for nm, wd in [("w0", wq), ("w1", wk), ("wvs", wv)]:
                wt = wpool.tile([P, 8, CG], BF16, tag="w", name=nm)
                nc.gpsimd.dma_start(wt, wd[:])
                w_tiles[nm] = wt
            wp_v = wpool.tile([P, 4, C], BF16, tag="w", name="wpv")
            nc.gpsimd.dma_start(wp_v, wp[:])

            for side, (wdram, dst) in enumerate([(wq, qT_sb), (wk, kT_sb)]):
                w_sb = w_tiles[f"w{side}"]
                for mt in range(4):
                    for nbh in range(2):
                        ps = psS.tile([P, 2, 512], F32, tag="psS")
                        for kt in range(8):
                            for nb2 in range(2):
                                nc.tensor.matmul(
                                    ps[:, nb2],
                                    w_sb[:, kt, mt * P:(mt + 1) * P],
                                    xT_bf[:, kt,
                                          nbh * 1024 + nb2 * 512:
                                          nbh * 1024 + (nb2 + 1) * 512],
                                    start=(kt == 0), stop=(kt == 7))
                        nc.vector.tensor_copy(
                            dst[:, mt, nbh * 1024:(nbh + 1) * 1024],
                            ps.rearrange("p a b -> p (a b)"))

            wv_sb = w_tiles["wvs"]
            for tt in range(16):
                ps = psU.tile([P, 512], F32, tag="psU", name=f"psV{tt}")
                for kt in range(8):
                    nc.tensor.matmul(
                        ps,
                        xT_bf[:, kt, tt * P:(tt + 1) * P],
                        wv_sb[:, kt, :],
                        start=(kt == 0), stop=(kt == 7))
                nc.scalar.copy(V_sb[:, tt, :, 0:D],
                               ps.rearrange("p (h d) -> p h d", h=HPG))
                nc.vector.memset(V_sb[:, tt, :, D:D + 1], 1.0)

            # ---- Phase C: attention (head pairs on PE row groups) ----
            yT_sb = big.tile([P, 4, T], BF16, tag="big")   # reuses xT slot
            # per-head ypad rows, stored at the head's partition range
            ypadA = misc.tile([P, 4, 512], F32, tag="ypadA")

            for hp in range(4):
                for parity in range(2):
                    h = 2 * hp + parity
                    psPad = psU.tile([D + 1, 512], F32, tag="psU",
                                     name=f"pad{h}")
                    for kt in range(16):
                        nc.tensor.matmul(
                            psPad[:, 0:1],
                            V_sb[:, kt, h, :],
                            uni_sb[:, 0:1],
                            start=(kt == 0), stop=(kt == 15))
                    if parity == 0:
                        nc.vector.tensor_copy(
                            ypadA[0:D, hp, :],
                            psPad[0:D, 0:1].to_broadcast([D, 512]))
                    else:
                        yptmp = norm.tile([D, 512], F32, tag="yptmp")
                        nc.vector.tensor_copy(
                            yptmp, psPad[0:D, 0:1].to_broadcast([D, 512]))
                        nc.sync.dma_start(ypadA[D:P, hp, :], yptmp)

                for j in range(4):
                    nkt = 4 * (j + 1)
                    Upr = [psU.tile([D + 1, 512], F32, tag="psU",
                                    name=f"U_{hp}_{j}_{par}")
                           for par in range(2)]

                    def s_exp(kt, j=j, hp=hp):
                        dlt = 128 * kt - 512 * j
                        c0 = max(dlt, 0)
                        ss = psS.tile([P, 2, 512], F32, tag="psS")
                        for parity in range(2):
                            p0 = parity * D
                            nc.tensor.matmul(
                                ss[:, parity, c0:512],
                                kT_sb[p0:p0 + D, hp, kt * P:(kt + 1) * P],
                                qT_sb[p0:p0 + D, hp,
                                      512 * j + c0:512 * (j + 1)],
                                start=True, stop=True)
                        pt = ptpool.tile([P, 2, 512], BF16, tag="pt")
                        if c0 > 0:
                            nc.vector.memset(pt[:, :, 0:c0], 0.0)
                        nc.scalar.activation(
                            pt[:, :, c0:512], ss[:, :, c0:512],
                            mybir.ActivationFunctionType.Exp,
                            bias=0.0, scale=SCALE)
                        if dlt >= 0:
                            nc.vector.tensor_mul(
                                out=pt[:, :, c0:c0 + P],
                                in0=pt[:, :, c0:c0 + P],
                                in1=m01_sb[:, None, :].to_broadcast(
                                    [P, 2, P]))
                        return pt

                    def pv(kt, pt, hp=hp):
                        for parity in range(2):
                            h = 2 * hp + parity
                            nc.tensor.matmul(
                                Upr[parity],
                                V_sb[:, kt, h, :],
                                pt[:, parity, :],
                                start=(kt == 0), stop=(kt == nkt - 1))

                    prev = None
                    for kt in range(nkt):
                        cur = s_exp(kt)
                        if prev is not None:
                            pv(kt - 1, prev)
                        prev = cur
                    pv(nkt - 1, prev)

                    # stash unnormalized y and the denominator row
                    for parity in range(2):
                        U = Upr[parity]
                        blk = slice(512 * j, 512 * (j + 1))
                        r = (hp * 2 + parity) * 4 + j
                        dtf = norm.tile([P, 512], F32, tag="dt")
                        nc.vector.tensor_copy(dtf[D:D + 1, :], U[D:D + 1, :])
                        nc.sync.dma_start(den_sb[r:r + 1, :],
                                          dtf[D:D + 1, :])
                        if parity == 0:
                            nc.vector.tensor_copy(yT_sb[0:D, hp, blk],
                                                  U[0:D, :])
                        else:
                            ytmp = norm.tile([D, 512], BF16, tag="ytmp")
                            nc.vector.tensor_copy(ytmp, U[0:D, :])
                            nc.sync.dma_start(yT_sb[D:P, hp, blk], ytmp)

            # ---- Phase C2: batched normalization + l-blend ----
            denq = misc.tile([32, 512], F32, tag="denq")
            nc.vector.reciprocal(denq, den_sb)
            nc.vector.tensor_mul(out=denq, in0=denq, in1=qmA_sb)
            dend = rdram.tile([32, 512], F32, tag="dend")
            nc.sync.dma_start(dend, denq)

            for hp in range(4):
                for j in range(4):
                    blk = slice(512 * j, 512 * (j + 1))
                    rb = norm.tile([P, 512], F32, tag="rb")
                    for parity in range(2):
                        r = (hp * 2 + parity) * 4 + j
                        row = dend[r:r + 1, :]
                        src = bass.AP(
                            tensor=row.tensor, offset=row.offset,
                            ap=[[0, D]] + list(row.ap[1:]))
                        nc.sync.dma_start(rb[parity * D:(parity + 1) * D, :],
                                          src)
                    t3 = norm.tile([P, 512], F32, tag="t3")
                    nc.vector.tensor_mul(
                        out=t3, in0=ypadA[:, hp, :], in1=qmn_sb[:, blk])
                    ys = yT_sb[:, hp, blk]
                    nc.vector.tensor_mul(out=ys, in0=ys, in1=rb)
                    nc.vector.tensor_add(out=ys, in0=ys, in1=t3)

            if debug:
                nc.gpsimd.dma_start(d_yT[:], yT_sb)

            # ---- Phase D: output projection ----
            for mt in range(8):
                psa = psS.tile([P, 2, 512], F32, tag="psS", name=f"po{mt}a")
                psb = psS.tile([P, 2, 512], F32, tag="psS", name=f"po{mt}b")
                outs = [psa[:, 0], psa[:, 1], psb[:, 0], psb[:, 1]]
                for ct in range(4):
                    for qb in range(4):
                        nc.tensor.matmul(
                            outs[qb],
                            wp_v[:, ct, mt * P:(mt + 1) * P],
                            yT_sb[:, ct, qb * 512:(qb + 1) * 512],
                            start=(ct == 0), stop=(ct == 3))
                for half, pp in enumerate([psa, psb]):
                    ot = obpool.tile([P, 1024], F32, tag="ob")
                    nc.scalar.copy(ot, pp.rearrange("p a b -> p (a b)"))
                    nc.sync.dma_start(
                        oT[mt * P:(mt + 1) * P,
                           half * 1024:(half + 1) * 1024], ot)

    nc.compile()
    return nc


def _prep_inputs(x, l, W_attn, b_attn, W_proj, b_proj):
    x = np.asarray(x, dtype=np.float32)
    W_attn = np.asarray(W_attn, dtype=np.float32)
    W_proj = np.asarray(W_proj, dtype=np.float32)
    lv = np.asarray(l).astype(np.int64)

    import ml_dtypes
    m01 = np.where(np.arange(P)[:, None] > np.arange(P)[None, :],
                   0.0, 1.0).astype(ml_dtypes.bfloat16)

    in_maps = []
    for b in range(B):
        xTb = np.ascontiguousarray(x[b].T)
        lb = int(np.clip(lv[b], 0, T))
        qrow = (np.arange(T) < lb).astype(np.float32)
        qmn = np.ascontiguousarray(np.broadcast_to(1.0 - qrow, (P, T))
                                   ).astype(np.float32)
        qmA = np.empty((32, 512), dtype=np.float32)
        for r in range(32):
            j = r % 4
            qmA[r] = qrow[512 * j:512 * (j + 1)]
        for g in range(2):
            cs = slice(g * CG, (g + 1) * CG)
            wqg = np.ascontiguousarray(
                W_attn[:, 0:C][:, cs].reshape(8, P, CG).transpose(1, 0, 2))
            wkg = np.ascontiguousarray(
                W_attn[:, C:2 * C][:, cs].reshape(8, P, CG).transpose(1, 0, 2))
            wvg = np.ascontiguousarray(
                W_attn[:, 2 * C:3 * C][:, cs].reshape(8, P, CG).transpose(1, 0, 2))
            wpg = np.ascontiguousarray(
                W_proj[cs, :].reshape(4, P, C).transpose(1, 0, 2))
            in_maps.append({
                "xT": xTb, "wq": wqg, "wk": wkg, "wv": wvg, "wp": wpg,
                "qmn": qmn, "qmA": qmA, "m01": m01,
            })
    return in_maps


def kernel(x, l, W_attn, b_attn, W_proj, b_proj, _want_profile=False):
    global _CACHED_NC
    if _CACHED_NC is None:
        _CACHED_NC = build_nc()
    nc = _CACHED_NC

    b_attn = np.asarray(b_attn, dtype=np.float32)
    b_proj = np.asarray(b_proj, dtype=np.float32)
    assert not np.any(b_attn), "nonzero b_attn not supported by this kernel"

    in_maps = _prep_inputs(x, l, W_attn, b_attn, W_proj, b_proj)
    res = run_bass_kernel_spmd(nc, in_maps, core_ids=list(range(8)),
                               trace=_want_profile)

    out = np.empty((B, T, C), dtype=np.float32)
    for b in range(B):
        acc = res.results[2 * b]["oT"] + res.results[2 * b + 1]["oT"]
        out[b] = acc.T + b_proj[None, :]
    if _want_profile:
        return out, res
    return out


# revision 17
# speedup vs baseline: 1.7117x; 1.0019x over previous
"""Trainium2 Bass kernel for nn_CausalSelfAttention_17368847745133.

Sharding (8 NeuronCores): core (b, g) = batch b in 0..3 x head-group g in
0..1 (8 heads each; Megatron column/row-parallel c_attn / c_proj).  The host
passes x[b].T so every device matmul runs transpose-free:

  qT/kT [512,2048] : matmul(lhsT=W_q|k slice, rhs=xT)      (transposed proj)
  V     [2048,512] : matmul(lhsT=xT tile, rhs=W_v slice)   (natural layout)
  S^T   [k,q]      : matmul(lhsT=kT head, rhs=qT head)     (d=64 contraction,
                     head pairs packed on PE row-groups 0-63 / 64-127)
  P^T   = exp((S^T + causal_mask) / 8)    masked lanes underflow to exact 0
  U'    [65,q]     : matmul(lhsT=[V_head|ones], rhs=P^T)   row 64 = denom
  y^T   = U'[0:64] * bcast(qm / denom) + ypad * (1 - qm)
  oT    [1024,2048]: matmul(lhsT=W_proj rows, rhs=y^T); host sums the two
                     group partials, transposes, adds b_proj.

Rows q >= l[b] reproduce the reference exactly: the reference's additive
-1e8 mask makes every logit in those rows quantize to -1e8, so its softmax
is exactly uniform and y = mean_k v.  We compute ypad = (1/2048) sum_k v
once per head and blend it by the 0/1 column mask (built on host from l).
All matmuls run in bf16 (inputs cast on load); softmax statistics and the
normalization stay fp32.  Reciprocals are batched for all 64 (head, block)
rows into one 32-partition DVE op.
"""

import numpy as np

import concourse.bass as bass
import concourse.mybir as mybir
import concourse.tile as tile
from concourse import bacc
from concourse.bass_utils import run_bass_kernel_spmd

P = 128
B, T, C = 4, 2048, 1024
H, D = 16, 64
G = 2
HPG = H // G     # 8 heads per core
CG = HPG * D     # 512 channels per group
NEG = -1e8
F32 = mybir.dt.float32
BF16 = mybir.dt.bfloat16
SCALE = 0.125    # 1/sqrt(64)

_CACHED_NC = None


def build_nc(debug=False):
    nc = bacc.Bacc(trn_type="TRN2", target_bir_lowering=False)

    xT = nc.dram_tensor("xT", [C, T], F32, kind="ExternalInput")
    wq = nc.dram_tensor("wq", [P, 8, CG], F32, kind="ExternalInput")
    wk = nc.dram_tensor("wk", [P, 8, CG], F32, kind="ExternalInput")
    wv = nc.dram_tensor("wv", [P, 8, CG], F32, kind="ExternalInput")
    wp = nc.dram_tensor("wp", [P, 4, C], F32, kind="ExternalInput")
    qmn = nc.dram_tensor("qmn", [P, T], F32, kind="ExternalInput")   # 1-qm
    qmA = nc.dram_tensor("qmA", [32, 512], F32, kind="ExternalInput")
    m01 = nc.dram_tensor("m01", [P, P], BF16, kind="ExternalInput")
    oT = nc.dram_tensor("oT", [C, T], F32, kind="ExternalOutput")
    if debug:
        d_yT = nc.dram_tensor("d_yT", [P, 4, T], F32, kind="ExternalOutput")

    with tile.TileContext(nc) as tc:
        with tc.tile_pool(name="big", bufs=1) as big, \
             tc.tile_pool(name="qk", bufs=1) as qkpool, \
             tc.tile_pool(name="vp", bufs=1) as vpool, \
             tc.tile_pool(name="w", bufs=2) as wpool, \
             tc.tile_pool(name="pt", bufs=4) as ptpool, \
             tc.tile_pool(name="misc", bufs=1) as misc, \
             tc.tile_pool(name="norm", bufs=2) as norm, \
             tc.tile_pool(name="ob", bufs=3) as obpool, \
             tc.tile_pool(name="rdram", bufs=2, space="DRAM") as rdram, \
             tc.tile_pool(name="psS", bufs=3, space="PSUM") as psS, \
             tc.tile_pool(name="psU", bufs=2, space="PSUM") as psU:

            # ---- constants ----
            qmn_sb = misc.tile([P, T], F32, tag="qmn")
            m01_sb = misc.tile([P, P], BF16, tag="m01")
            qmA_sb = misc.tile([32, 512], F32, tag="qmA")
            uni_sb = misc.tile([P, 2], BF16, tag="uni")
            nc.sync.dma_start(qmn_sb, qmn[:])
            nc.sync.dma_start(m01_sb, m01[:])
            nc.sync.dma_start(qmA_sb, qmA[:])
            nc.vector.memset(uni_sb, 1.0 / T)
            # all (head, block) softmax denominators, gathered by small DMAs
            den_sb = misc.tile([32, 512], F32, tag="den")

            # ---- Phase B: QKV projections (bf16, casting DMA loads) ----
            xT_bf = big.tile([P, 8, T], BF16, tag="big")
            for ct in range(8):
                nc.gpsimd.dma_start(xT_bf[:, ct], xT[ct * P:(ct + 1) * P, :])

            qT_sb = qkpool.tile([P, 4, T], BF16, tag="qT")
            kT_sb = qkpool.tile([P, 4, T], BF16, tag="kT")
            V_sb = vpool.tile([P, 16, HPG, D + 1], BF16, tag="V")

            w_tiles = {}
            for nm, wd in [("w0", wq), ("w1", wk), ("wvs", wv)]:
                wt = wpool.tile([P, 8, CG], BF16, tag="w", name=nm)
                nc.gpsimd.dma_start(wt, wd[:])
                w_tiles[nm] = wt
            wp_v = wpool.tile([P, 4, C], BF16, tag="w", name="wpv")
            nc.gpsimd.dma_start(wp_v, wp[:])

            for mt in range(4):
                for side, dst in [(0, qT_sb), (1, kT_sb)]:
                    w_sb = w_tiles[f"w{side}"]
                    for nbh in range(2):
                        ps = psS.tile([P, 2, 512], F32, tag="psS")
                        for kt in range(8):
                            for nb2 in range(2):
                                nc.tensor.matmul(
                                    ps[:, nb2],
                                    w_sb[:, kt, mt * P:(mt + 1) * P],
                                    xT_bf[:, kt,
                                          nbh * 1024 + nb2 * 512:
                                          nbh * 1024 + (nb2 + 1) * 512],
                                    start=(kt == 0), stop=(kt == 7))
                        nc.vector.tensor_copy(
                            dst[:, mt, nbh * 1024:(nbh + 1) * 1024],
                            ps.rearrange("p a b -> p (a b)"))

            wv_sb = w_tiles["wvs"]
            for tt in range(16):
                ps = psU.tile([P, 512], F32, tag="psU", name=f"psV{tt}")
                for kt in range(8):
                    nc.tensor.matmul(
                        ps,
                        xT_bf[:, kt, tt * P:(tt + 1) * P],
                        wv_sb[:, kt, :],
                        start=(kt == 0), stop=(kt == 7))
                nc.scalar.copy(V_sb[:, tt, :, 0:D],
                               ps.rearrange("p (h d) -> p h d", h=HPG))
                nc.vector.memset(V_sb[:, tt, :, D:D + 1], 1.0)

            # ---- Phase C: attention (head pairs on PE row groups) ----
            yT_sb = big.tile([P, 4, T], BF16, tag="big")   # reuses xT slot
            # per-head ypad rows, stored at the head's partition range
            ypadA = misc.tile([P, 4, 512], F32, tag="ypadA")

            for hp in range(4):
                for j in range(4):
                    nkt = 4 * (j + 1)
                    Upr = [psU.tile([D + 1, 512], F32, tag="psU",
                                    name=f"U_{hp}_{j}_{par}")
                           for par in range(2)]

                    def s_exp(kt, j=j, hp=hp):
                        dlt = 128 * kt - 512 * j
                        c0 = max(dlt, 0)
                        ss = psS.tile([P, 2, 512], F32, tag="psS")
                        for parity in range(2):
                            p0 = parity * D
                            nc.tensor.matmul(
                                ss[:, parity, c0:512],
                                kT_sb[p0:p0 + D, hp, kt * P:(kt + 1) * P],
                                qT_sb[p0:p0 + D, hp,
                                      512 * j + c0:512 * (j + 1)],
                                start=True, stop=True)
                        pt = ptpool.tile([P, 2, 512], BF16, tag="pt")
                        if c0 > 0:
                            nc.vector.memset(pt[:, :, 0:c0], 0.0)
                        nc.scalar.activation(
                            pt[:, :, c0:512], ss[:, :, c0:512],
                            mybir.ActivationFunctionType.Exp,
                            bias=0.0, scale=SCALE)
                        if dlt >= 0:
                            nc.vector.tensor_mul(
                                out=pt[:, :, c0:c0 + P],
                                in0=pt[:, :, c0:c0 + P],
                                in1=m01_sb[:, None, :].to_broadcast(
                                    [P, 2, P]))
                        return pt

                    def pv(kt, pt, hp=hp):
                        for parity in range(2):
                            h = 2 * hp + parity
                            nc.tensor.matmul(
                                Upr[parity],
                                V_sb[:, kt, h, :],
                                pt[:, parity, :],
                                start=(kt == 0), stop=(kt == nkt - 1))

                    prev = None
                    for kt in range(nkt):
                        cur = s_exp(kt)
                        if prev is not None:
                            pv(kt - 1, prev)
                        prev = cur
                    pv(nkt - 1, prev)

                    # stash unnormalized y and the denominator row
                    for parity in range(2):
                        U = Upr[parity]
                        blk = slice(512 * j, 512 * (j + 1))
                        r = (hp * 2 + parity) * 4 + j
                        dtf = norm.tile([P, 512], F32, tag="dt")
                        nc.vector.tensor_copy(dtf[D:D + 1, :], U[D:D + 1, :])
                        nc.sync.dma_start(den_sb[r:r + 1, :],
                                          dtf[D:D + 1, :])
                        if parity == 0:
                            nc.vector.tensor_copy(yT_sb[0:D, hp, blk],
                                                  U[0:D, :])
                        else:
                            ytmp = norm.tile([D, 512], BF16, tag="ytmp")
                            nc.vector.tensor_copy(ytmp, U[0:D, :])
                            nc.sync.dma_start(yT_sb[D:P, hp, blk], ytmp)

            # pad rows: ypad_h = (1/T) sum_k v, for every head
            for hp in range(4):
                for parity in range(2):
                    h = 2 * hp + parity
                    psPad = psU.tile([D + 1, 512], F32, tag="psU",
                                     name=f"pad{h}")
                    for kt in range(16):
                        nc.tensor.matmul(
                            psPad[:, 0:1],
                            V_sb[:, kt, h, :],
                            uni_sb[:, 0:1],
                            start=(kt == 0), stop=(kt == 15))
                    if parity == 0:
                        nc.vector.tensor_copy(
                            ypadA[0:D, hp, :],
                            psPad[0:D, 0:1].to_broadcast([D, 512]))
                    else:
                        yptmp = norm.tile([D, 512], F32, tag="yptmp")
                        nc.vector.tensor_copy(
                            yptmp, psPad[0:D, 0:1].to_broadcast([D, 512]))
                        nc.sync.dma_start(ypadA[D:P, hp, :], yptmp)

            # ---- Phase C2: batched normalization + l-blend ----
            denq = misc.tile([32, 512], F32, tag="denq")
            nc.vector.reciprocal(denq, den_sb)
            nc.vector.tensor_mul(out=denq, in0=denq, in1=qmA_sb)
            dend = rdram.tile([32, 512], F32, tag="dend")
            nc.sync.dma_start(dend, denq)

            for j in range(4):
                for hp in range(4):
                    blk = slice(512 * j, 512 * (j + 1))
                    rb = norm.tile([P, 512], F32, tag="rb")
                    for parity in range(2):
                        r = (hp * 2 + parity) * 4 + j
                        row = dend[r:r + 1, :]
                        src = bass.AP(
                            tensor=row.tensor, offset=row.offset,
                            ap=[[0, D]] + list(row.ap[1:]))
                        nc.sync.dma_start(rb[parity * D:(parity + 1) * D, :],
                                          src)
                    t3 = norm.tile([P, 512], F32, tag="t3")
                    nc.vector.tensor_mul(
                        out=t3, in0=ypadA[:, hp, :], in1=qmn_sb[:, blk])
                    ys = yT_sb[:, hp, blk]
                    nc.vector.tensor_mul(out=ys, in0=ys, in1=rb)
                    nc.vector.tensor_add(out=ys, in0=ys, in1=t3)

            if debug:
                nc.gpsimd.dma_start(d_yT[:], yT_sb)

            # ---- Phase D: output projection ----
            for mt in range(8):
                psa = psS.tile([P, 2, 512], F32, tag="psS", name=f"po{mt}a")
                psb = psS.tile([P, 2, 512], F32, tag="psS", name=f"po{mt}b")
                outs = [psa[:, 0], psa[:, 1], psb[:, 0], psb[:, 1]]
                for ct in range(4):
                    for qb in range(4):
                        nc.tensor.matmul(
                            outs[qb],
                            wp_v[:, ct, mt * P:(mt + 1) * P],
                            yT_sb[:, ct, qb * 512:(qb + 1) * 512],
                            start=(ct == 0), stop=(ct == 3))
                for half, pp in enumerate([psa, psb]):
                    ot = obpool.tile([P, 1024], F32, tag="ob")
                    nc.vector.tensor_copy(ot, pp.rearrange("p a b -> p (a b)"))
                    nc.sync.dma_start(
                        oT[mt * P:(mt + 1) * P,
                           half * 1024:(half + 1) * 1024], ot)

    nc.compile()
    return nc


def _prep_inputs(x, l, W_attn, b_attn, W_proj, b_proj):
    x = np.asarray(x, dtype=np.float32)
    W_attn = np.asarray(W_attn, dtype=np.float32)
    W_proj = np.asarray(W_proj, dtype=np.float32)
    lv = np.asarray(l).astype(np.int64)

    import ml_dtypes
    m01 = np.where(np.arange(P)[:, None] > np.arange(P)[None, :],
                   0.0, 1.0).astype(ml_dtypes.bfloat16)

    in_maps = []
    for b in range(B):
        xTb = np.ascontiguousarray(x[b].T)
        lb = int(np.clip(lv[b], 0, T))
        qrow = (np.arange(T) < lb).astype(np.float32)
        qmn = np.ascontiguousarray(np.broadcast_to(1.0 - qrow, (P, T))
                                   ).astype(np.float32)
        qmA = np.empty((32, 512), dtype=np.float32)
        for r in range(32):
            j = r % 4
            qmA[r] = qrow[512 * j:512 * (j + 1)]
        for g in range(2):
            cs = slice(g * CG, (g + 1) * CG)
            wqg = np.ascontiguousarray(
                W_attn[:, 0:C][:, cs].reshape(8, P, CG).transpose(1, 0, 2))
            wkg = np.ascontiguousarray(
                W_attn[:, C:2 * C][:, cs].reshape(8, P, CG).transpose(1, 0, 2))
            wvg = np.ascontiguousarray(
                W_attn[:, 2 * C:3 * C][:, cs].reshape(8, P, CG).transpose(1, 0, 2))
            wpg = np.ascontiguousarray(
                W_proj[cs, :].reshape(4, P, C).transpose(1, 0, 2))
            in_maps.append({
                "xT": xTb, "wq": wqg, "wk": wkg, "wv": wvg, "wp": wpg,
                "qmn": qmn, "qmA": qmA, "m01": m01,
            })
    return in_maps


def kernel(x, l, W_attn, b_attn, W_proj, b_proj, _want_profile=False):
    global _CACHED_NC
    if _CACHED_NC is None:
        _CACHED_NC = build_nc()
    nc = _CACHED_NC

    b_attn = np.asarray(b_attn, dtype=np.float32)
    b_proj = np.asarray(b_proj, dtype=np.float32)
    assert not np.any(b_attn), "nonzero b_attn not supported by this kernel"

    in_maps = _prep_inputs(x, l, W_attn, b_attn, W_proj, b_proj)
    res = run_bass_kernel_spmd(nc, in_maps, core_ids=list(range(8)),
                               trace=_want_profile)

    out = np.empty((B, T, C), dtype=np.float32)
    for b in range(B):
        acc = res.results[2 * b]["oT"] + res.results[2 * b + 1]["oT"]
        out[b] = acc.T + b_proj[None, :]
    if _want_profile:
        return out, res
    return out
